# revision 49
# baseline (speedup 1.0000x reference)
"""Multi-head causal attention with RoPE on 8 Trainium2 NeuronCores.

Sharding: data-parallel over batch (2) x tensor-parallel over heads (16 -> 4
per core). Each core computes q/k/v projections for its 4 heads on its batch
element, attention, and a partial output projection (its rows of wo); the
host sums the 4 partials per batch element.

Device-side layout trick: everything is computed "transposed" (scores held as
[t, s]) so no on-device transposes are needed anywhere:
  - host supplies x^T (bf16), so projections produce q^T/k^T [head_dim, s]
    directly and v [t, head_dim] directly
  - softmax denominators come from an all-ones matmul (partition reduction on
    the tensor engine); 1/x is exp(-ln(x)) on the scalar engine (one ACT
    table set for exp+ln)
  - attn^T [n, s] is exactly the lhsT the wo matmul wants
RoPE pair-swap is done by permuting the wq/wk columns on the host into
(even|odd) half-layout so the swap becomes two partition-halved SBUF->SBUF
DMA copies instead of cross-lane compute.

kernel() inspects the mask and dispatches to one of three compiled variants:
causal (block-skipping + const triangular mask), full attention with no mask,
or full attention with an arbitrary additive mask (added via identity-matmul
accumulation into the scores PSUM).
"""

import math

import ml_dtypes
import numpy as np

import concourse.bass as bass
import concourse.mybir as mybir
import concourse.tile as tile
from concourse import bacc
from concourse.bass_utils import run_bass_kernel_spmd

BF16 = ml_dtypes.bfloat16
F32 = mybir.dt.float32
BF = mybir.dt.bfloat16
AF = mybir.ActivationFunctionType

N_CORES = 8
B = 2
S = 2048
D = 2048
H = 16
HD = 128
H_LOC = 4          # heads per core
N_LOC = H_LOC * HD  # 512 local head dims
NJ = 4             # s-chunks
SC = S // NJ       # 512 s-chunk width
DCH = D // 128     # 16 contraction chunks
SCALE = 1.0 / math.sqrt(HD)

_BUILDS: dict = {}
LAST_RESULT = None


def _build(variant: str, nj: int = NJ, do_attn: bool = True,
           do_wo: bool = True):
    """variant in {'causal', 'full_nomask', 'full_mask'}"""
    causal = variant == "causal"
    use_mask = variant == "full_mask"

    nc = bacc.Bacc("TRN2", target_bir_lowering=False, debug=False,
                   num_devices=N_CORES)

    xt_d = nc.dram_tensor("xt", [NJ, 128, DCH, SC], BF, kind="ExternalInput").ap()
    wq_d = nc.dram_tensor("wq", [128, DCH, N_LOC], BF, kind="ExternalInput").ap()
    wk_d = nc.dram_tensor("wk", [128, DCH, N_LOC], BF, kind="ExternalInput").ap()
    wv_d = nc.dram_tensor("wv", [128, DCH, N_LOC], BF, kind="ExternalInput").ap()
    wo_d = nc.dram_tensor("wo", [128, H_LOC, D], BF, kind="ExternalInput").ap()
    cose_d = nc.dram_tensor("cose", [128, S], F32, kind="ExternalInput").ap()
    sine_d = nc.dram_tensor("sine", [128, S], F32, kind="ExternalInput").ap()
    tri_d = None
    maskt_d = None
    if causal:
        tri_d = nc.dram_tensor("tri4", [128, 4, SC], BF, kind="ExternalInput").ap()
    if use_mask:
        maskt_d = nc.dram_tensor("maskt", [NJ, 128, DCH, SC], BF,
                                 kind="ExternalInput").ap()
    out_d = nc.dram_tensor("out", [S, D], F32, kind="ExternalOutput").ap()
    out_v = out_d.rearrange("(a p) d -> a p d", p=128)

    with tile.TileContext(nc) as tc:
        with (
            tc.tile_pool(name="singles", bufs=1) as singles,
            tc.tile_pool(name="doubles", bufs=2) as doubles,
            tc.tile_pool(name="triples", bufs=3) as triples,
            tc.tile_pool(name="quads", bufs=4) as quads,
            tc.tile_pool(name="ps1", bufs=1, space="PSUM") as ps1,
            tc.tile_pool(name="ps2", bufs=2, space="PSUM") as ps2,
        ):
            rope_pool = quads if causal else (doubles if use_mask else triples)
            stage_pool = doubles if use_mask else triples
            # ---- constants / persistent tensors ----
            wq_sb = singles.tile([128, DCH, N_LOC], BF, tag="wq")
            wk_sb = singles.tile([128, DCH, N_LOC], BF, tag="wk")
            wv_sb = singles.tile([128, DCH, N_LOC], BF, tag="wv")
            wo_sb = singles.tile([128, H_LOC, D], BF, tag="wo")
            # piecewise loads in consumption order so the first matmuls start
            # as soon as their slice lands (sub-tile dep tracking)
            xt_pool = singles if use_mask else doubles
            xt0_sb = xt_pool.tile([128, DCH, SC], BF, tag="xt")
            nc.sync.dma_start(out=wq_sb[:], in_=wq_d[:])
            nc.gpsimd.dma_start(out=xt0_sb[:], in_=xt_d[0])
            cose_sb = singles.tile([128, S], F32, tag="cose")
            sine_sb = singles.tile([128, S], F32, tag="sine")
            nc.sync.dma_start(out=cose_sb[:], in_=cose_d[:])
            nc.sync.dma_start(out=sine_sb[:], in_=sine_d[:])
            nc.sync.dma_start(out=wk_sb[:], in_=wk_d[:])
            nc.sync.dma_start(out=wv_sb[:], in_=wv_d[:])
            nc.sync.dma_start(out=wo_sb[:], in_=wo_d[:])
            from concourse.masks import make_identity
            ones_sb = singles.tile([128, 128], BF, tag="ones")
            nc.vector.memset(ones_sb[:], 1.0)
            ident_sb = singles.tile([128, 128], BF, tag="ident")
            make_identity(nc, ident_sb[:])
            tri_sb = None
            if causal:
                tri_sb = singles.tile([128, 4, SC], BF, tag="tri")
                nc.sync.dma_start(out=tri_sb[:], in_=tri_d[:])
            # k^T (rotated) and v accumulate across s-chunks
            ktrot = singles.tile([128, H_LOC, S], BF, tag="ktrot")
            v_sb = singles.tile([128, NJ * H_LOC, SC], BF, tag="v")
            qtrot_all = None
            if not causal:
                qtrot_all = singles.tile([128, H_LOC, S], BF, tag="qtrot_all")

            def projections(j, qdest, qsl, xt_pre=None):
                js = j * SC
                if xt_pre is not None:
                    xt_sb = xt_pre
                else:
                    xt_sb = xt_pool.tile([128, DCH, SC], BF, tag="xt")
                    nc.gpsimd.dma_start(out=xt_sb[:], in_=xt_d[j])

                # ---- q/k projections + rope ----
                # head-pairs with d-outer matmuls (piece consumption matches
                # DMA arrival); rope in two phases so the TT3 adds (which wait
                # on the swap-DMA round trip) never head-of-line-block the
                # TT1/TT2 muls that release the PSUM accumulators
                for w_sb, dest, dsl in ((wq_sb, qdest, qsl),
                                        (wk_sb, ktrot, slice(js, js + SC))):
                    for hp in range(2):
                        parts = []
                        for hh in range(2):
                            h = hp * 2 + hh
                            ps = ps2.tile([128, SC], F32, tag="qkv1")
                            for d in range(DCH):
                                nc.tensor.matmul(
                                    ps[:],
                                    w_sb[:, d, h * 128:(h + 1) * 128],
                                    xt_sb[:, d, :],
                                    start=(d == 0), stop=(d == DCH - 1),
                                )
                            a_sb = rope_pool.tile([128, SC], F32, tag="ropeA")
                            nc.vector.tensor_mul(
                                a_sb[:], ps[:], cose_sb[:, js:js + SC])
                            b_sb = rope_pool.tile([128, SC], F32, tag="ropeB")
                            nc.vector.tensor_mul(
                                b_sb[:], ps[:], sine_sb[:, js:js + SC])
                            # half-swap via SBUF->SBUF DMA (cross-partition)
                            b2_sb = rope_pool.tile([128, SC], F32, tag="ropeB2")
                            nc.scalar.dma_start(out=b2_sb[0:64, :],
                                                in_=b_sb[64:128, :])
                            nc.scalar.dma_start(out=b2_sb[64:128, :],
                                                in_=b_sb[0:64, :])
                            parts.append((h, a_sb, b2_sb))
                        for h, a_sb, b2_sb in parts:
                            if dsl is None:
                                dst = dest[:, h, :]
                            else:
                                dst = dest[:, h, dsl]
                            nc.vector.tensor_add(dst, a_sb[:], b2_sb[:])

                # ---- v projection ----
                for tl in range(4):
                    ps = ps2.tile([128, SC], F32, tag="qkv1")
                    for d in range(DCH):
                        nc.tensor.matmul(
                            ps[:],
                            xt_sb[:, d, tl * 128:(tl + 1) * 128],
                            wv_sb[:, d, :],
                            start=(d == 0), stop=(d == DCH - 1),
                        )
                    nc.scalar.copy(out=v_sb[:, 4 * j + tl, :], in_=ps[:])

            def wo_units(j, attnT_j, st, dcs):
                for dc in dcs:
                    wps = ps2.tile([128, SC], F32, tag="qkv1")
                    for h2 in range(H_LOC):
                        nc.tensor.matmul(
                            wps[:],
                            attnT_j[:, h2, st * 128:(st + 1) * 128],
                            wo_sb[:, h2, dc * SC:(dc + 1) * SC],
                            start=(h2 == 0), stop=(h2 == H_LOC - 1),
                        )
                    o_sb = stage_pool.tile([128, SC], F32, tag="ostage")
                    if (st + dc) % 2 == 0:
                        nc.scalar.copy(out=o_sb[:], in_=wps[:])
                    else:
                        nc.vector.tensor_copy(o_sb[:], wps[:])
                    nc.sync.dma_start(
                        out=out_v[4 * j + st][:, dc * SC:(dc + 1) * SC],
                        in_=o_sb[:])

            def attention_and_wo(j, qtrot_h, prev=None):
                """qtrot_h(h) -> [128, SC] rhs AP for head h of chunk j."""
                js = j * SC
                maskt_sb = None
                if use_mask:
                    # reuses the "xt" slot (same shape; xt is done by pass 2)
                    maskt_sb = xt_pool.tile([128, DCH, SC], BF, tag="xt")
                    nc.sync.dma_start(out=maskt_sb[:], in_=maskt_d[j])

                n_tt = 4 * (j + 1) if causal else DCH
                attnT_j = doubles.tile([128, H_LOC, SC], BF, tag="attnT")
                if not do_attn:
                    for h in range(H_LOC):
                        nc.vector.tensor_copy(attnT_j[:, h, :],
                                              ktrot[:, h, j * SC:(j + 1) * SC])
                # diagonal pair-groups first: their longer mask+exp chain then
                # overlaps the remaining full groups' matmuls
                if causal:
                    pg_order = list(range(2 * j, 2 * j + 2)) + list(range(2 * j))
                else:
                    pg_order = list(range(n_tt // 2))
                for h in range(H_LOC if do_attn else 0):
                    sums_ps = ps1.tile([128, SC], F32, tag="sums")
                    pv_ps = ps1.tile([128, SC], F32, tag="pv")
                    for gi, pg in enumerate(pg_order):
                        sc_ps = ps2.tile([128, 2, SC], F32, tag="sc")
                        exp_sb = stage_pool.tile([128, 2, SC], BF, tag="exp")
                        diag = causal and pg >= 2 * j
                        for i_ in range(2):
                            tt = pg * 2 + i_
                            extra = diag or use_mask
                            nc.tensor.matmul(
                                sc_ps[:, i_, :],
                                ktrot[:, h, tt * 128:(tt + 1) * 128],
                                qtrot_h(h),
                                start=True, stop=not extra,
                            )
                            if diag:
                                # accumulate the causal mask tile via identity
                                # matmul: keeps the whole mask+exp chain off DVE
                                p = tt - 4 * j
                                nc.tensor.matmul(
                                    sc_ps[:, i_, :], ident_sb[:],
                                    tri_sb[:, p, :],
                                    start=False, stop=True,
                                )
                            elif use_mask:
                                nc.tensor.matmul(
                                    sc_ps[:, i_, :], ident_sb[:],
                                    maskt_sb[:, tt, :],
                                    start=False, stop=True,
                                )
                        nc.scalar.activation(out=exp_sb[:], in_=sc_ps[:],
                                             func=AF.Exp, scale=SCALE)
                        for i_ in range(2):
                            tt = pg * 2 + i_
                            first = gi == 0 and i_ == 0
                            last = gi == len(pg_order) - 1 and i_ == 1
                            nc.tensor.matmul(sums_ps[:], ones_sb[:],
                                             exp_sb[:, i_, :],
                                             start=first, stop=last)
                            nc.tensor.matmul(pv_ps[:],
                                             v_sb[:, tt, h * 128:(h + 1) * 128],
                                             exp_sb[:, i_, :],
                                             start=first, stop=last)
                    recip_sb = doubles.tile([128, SC], F32, tag="recip")
                    nc.vector.reciprocal_approx_fast(out=recip_sb[:], in_=sums_ps[:])
                    nc.vector.tensor_mul(attnT_j[:, h, :], pv_ps[:], recip_sb[:])
                    if prev is not None and do_wo:
                        # previous chunk's wo units fill the recip/exp latency
                        # bubble at the head boundary
                        wo_units(j - 1, prev, h, range(4))
                return attnT_j

            # wo for chunk j runs interleaved with attention of chunk j+1, so
            # its ACT copies and PE matmuls never delay the exp-critical path
            pending = None
            if causal:
                for j in range(nj):
                    qtrot = doubles.tile([128, H_LOC, SC], BF, tag="qtrot")
                    projections(j, qtrot, None, xt_pre=xt0_sb if j == 0 else None)
                    pending = attention_and_wo(j, lambda h, q=qtrot: q[:, h, :],
                                               prev=pending)
            else:
                for j in range(nj):
                    projections(j, qtrot_all, slice(j * SC, (j + 1) * SC),
                                xt_pre=xt0_sb if j == 0 else None)
                for j in range(nj):
                    js = j * SC
                    pending = attention_and_wo(
                        j, lambda h, js=js: qtrot_all[:, h, js:js + SC],
                        prev=pending)
            if pending is not None and do_wo:
                for st in range(4):
                    wo_units(nj - 1, pending, st, range(4))

    nc.compile()
    return nc


def _get_build(variant):
    if variant not in _BUILDS:
        _BUILDS[variant] = _build(variant)
    return _BUILDS[variant]


def _classify_mask(mask):
    if not np.any(mask):
        return "full_nomask"
    tril = np.tril(np.ones((S, S), dtype=bool))
    if np.all(mask[tril] == 0.0) and np.all(mask[~tril] <= -1e9):
        return "causal"
    return "full_mask"


def kernel(x, wq, wk, wv, wo, freqs_cos, freqs_sin, mask):
    global LAST_RESULT
    x = np.asarray(x)
    wq, wk, wv, wo = (np.asarray(w) for w in (wq, wk, wv, wo))
    freqs_cos = np.asarray(freqs_cos, dtype=np.float32)
    freqs_sin = np.asarray(freqs_sin, dtype=np.float32)
    mask = np.asarray(mask, dtype=np.float32)

    variant = _classify_mask(mask)
    nc = _get_build(variant)

    # half-layout column permutation within each head (even indices then odd)
    perm = np.concatenate([np.arange(0, 128, 2), np.arange(1, 128, 2)])

    def wproj_arr(w, g):
        cols = w[:, 512 * g:512 * (g + 1)].reshape(D, H_LOC, 128)
        cols = cols[:, :, perm].reshape(D, N_LOC)
        return np.ascontiguousarray(
            cols.reshape(DCH, 128, N_LOC).transpose(1, 0, 2)).astype(BF16)

    def wv_arr(w, g):
        cols = w[:, 512 * g:512 * (g + 1)]
        return np.ascontiguousarray(
            cols.reshape(DCH, 128, N_LOC).transpose(1, 0, 2)).astype(BF16)

    def wo_arr(g):
        rows = wo[512 * g:512 * (g + 1), :]
        return np.ascontiguousarray(
            rows.reshape(H_LOC, 128, D).transpose(1, 0, 2)).astype(BF16)

    # cos/sin in half-layout: rows j and j+64 carry pair j's cos; sine rows
    # 0..63 = +sin (source a_j -> target j+64), rows 64..127 = -sin
    cosE = np.empty((128, S), np.float32)
    sinE = np.empty((128, S), np.float32)
    cosE[0:64] = freqs_cos.T
    cosE[64:128] = freqs_cos.T
    sinE[0:64] = freqs_sin.T
    sinE[64:128] = -freqs_sin.T

    xt_b = []
    for b in range(B):
        xT = x[b].T.astype(BF16)  # [D, S]
        xt = np.ascontiguousarray(
            xT.reshape(DCH, 128, NJ, SC).transpose(2, 1, 0, 3))
        xt_b.append(xt)

    tri = None
    if variant == "causal":
        t_idx = np.arange(128)[:, None, None]
        p_idx = np.arange(4)[None, :, None]
        s_idx = np.arange(SC)[None, None, :]
        tri = np.where(t_idx + 128 * p_idx <= s_idx, 0.0, -1e30).astype(BF16)

    maskt = None
    if variant == "full_mask":
        # exp computes exp(SCALE * (scores + m')) with m' = mask^T / SCALE
        mT = (mask.T / SCALE).astype(BF16)  # [t, s]
        maskt = np.ascontiguousarray(
            mT.reshape(DCH, 128, NJ, SC).transpose(2, 1, 0, 3))

    wq_g = [wproj_arr(wq, g) for g in range(H_LOC)]
    wk_g = [wproj_arr(wk, g) for g in range(H_LOC)]
    wv_g = [wv_arr(wv, g) for g in range(H_LOC)]
    wo_g = [wo_arr(g) for g in range(H_LOC)]

    in_maps = []
    for c in range(N_CORES):
        b, g = c // 4, c % 4
        m = {
            "xt": xt_b[b],
            "wq": wq_g[g], "wk": wk_g[g], "wv": wv_g[g], "wo": wo_g[g],
            "cose": cosE, "sine": sinE,
        }
        if tri is not None:
            m["tri4"] = tri
        if maskt is not None:
            m["maskt"] = maskt
        in_maps.append(m)

    res = run_bass_kernel_spmd(nc, in_maps, list(range(N_CORES)))
    LAST_RESULT = res
    outs = [res.results[c]["out"] for c in range(N_CORES)]
    out = np.stack([
        outs[0] + outs[1] + outs[2] + outs[3],
        outs[4] + outs[5] + outs[6] + outs[7],
    ]).astype(np.float32)
    return out
